# revision 21
# baseline (speedup 1.0000x reference)
"""Dual-RoPE attention block (B=8, S=1024, 16 heads x 64) on 8 NeuronCores.

Sharding: data-parallel over batch, one batch element per core.

v2: software-pipelined emission so the Scalar engine (exp softmax, the
~324us/core floor) runs continuously from ~15us, and the PE never idles
long enough to drop out of the HAM 8/8 clock state.

Per-core dataflow (all matmuls bf16 inputs, fp32 PSUM accumulation):
  - Q,K projected in head-transposed layout [c, s]; rotate_half via
    partition-swapped SBUF strip DMAs; RoPE = 3 vector ops per tile with
    presigned sin rows.
  - V projected in row layout [s, d]; each head's 65-column block in vext
    is [2.0-const | v], so the PV matmul's row 0 yields 2*sum_k(exp) and
    pass-averaging folds into normalization.
  - scores matmul pairs use 64-row PE tiles (auto tile_position) so both
    heads' scores stream concurrently through the array.
  - PV PSUM evacuated by strip DMAs (not DVE) so the PSUM slots recycle
    off the vector engine's critical path; normalization (reciprocal,
    gpsimd broadcast, multiply, pass-combine) runs downstream on SBUF.
  - emission interleaves per kc-clock: scores pair, 2 exps, previous
    step's PV, and occasional projection bursts for the next head-pair.
"""

import numpy as np
import ml_dtypes

B, S, DM = 8, 1024, 1024
NH, HD = 16, 64
HD1 = HD + 1
NC = 8                # cores

_CACHE = {}


def _build():
    key = ("v2",)
    if key in _CACHE:
        return _CACHE[key]
    from concourse import bacc, mybir
    import concourse.tile as tile

    f32 = mybir.dt.float32
    bf16 = mybir.dt.bfloat16
    EXP = mybir.ActivationFunctionType.Exp

    nc = bacc.Bacc("TRN2", target_bir_lowering=False, debug=False,
                   enable_asserts=False, num_devices=NC)

    xT_d = nc.dram_tensor("xT", [DM, S], bf16, kind="ExternalInput").ap()
    wqkA_d = nc.dram_tensor("wqkA", [DM, 256], bf16, kind="ExternalInput").ap()
    wqkB_d = nc.dram_tensor("wqkB", [DM, 2 * DM - 256], bf16,
                            kind="ExternalInput").ap()
    wvT_d = nc.dram_tensor("wvT", [DM, DM], bf16, kind="ExternalInput").ap()
    woT_d = nc.dram_tensor("woT", [DM, DM], bf16, kind="ExternalInput").ap()
    trigC_d = nc.dram_tensor("trigC", [2, 128, S], bf16, kind="ExternalInput").ap()
    trigS_d = nc.dram_tensor("trigS", [2, 128, S], bf16, kind="ExternalInput").ap()
    out_d = nc.dram_tensor("out", [S, DM], f32, kind="ExternalOutput").ap()

    with tile.TileContext(nc) as tc:
        with (
            tc.tile_pool(name="persist", bufs=1) as pp,
            tc.tile_pool(name="work", bufs=1) as wk,
            tc.tile_pool(name="psg", bufs=1, space="PSUM") as pg,
        ):
            # ---------------- persistent tiles + input DMAs ----------------
            trigC_t = [pp.tile([128, S], bf16, name=f"trigC{p}") for p in range(2)]
            trigS_t = [pp.tile([128, S], bf16, name=f"trigS{p}") for p in range(2)]
            for p in range(2):
                nc.sync.dma_start(trigC_t[p][:], trigC_d[p])
                nc.sync.dma_start(trigS_t[p][:], trigS_d[p])

            xT_sb = [pp.tile([128, S], bf16, name=f"xT{i}") for i in range(8)]
            wqkA_sb = [pp.tile([128, 256], bf16, name=f"wqkA{i}")
                       for i in range(8)]
            wqkB_sb = [pp.tile([128, 2 * DM - 256], bf16, name=f"wqkB{i}")
                       for i in range(8)]
            wvT_sb = [pp.tile([128, DM], bf16, name=f"wv{i}") for i in range(8)]
            woT_sb = [pp.tile([128, DM], bf16, name=f"woT{i}") for i in range(8)]
            # DMA priority: xT + first-chunk weights, then V weights, then
            # the remaining qk weights, output weights last.
            for i in range(8):
                nc.sync.dma_start(xT_sb[i][:], xT_d[i * 128:(i + 1) * 128, :])
                nc.sync.dma_start(wqkA_sb[i][:], wqkA_d[i * 128:(i + 1) * 128, :])
            for i in range(8):
                nc.sync.dma_start(wvT_sb[i][:], wvT_d[i * 128:(i + 1) * 128, :])
            for i in range(8):
                nc.sync.dma_start(wqkB_sb[i][:], wqkB_d[i * 128:(i + 1) * 128, :])
            for i in range(8):
                nc.sync.dma_start(woT_sb[i][:], woT_d[i * 128:(i + 1) * 128, :])

            vext = [pp.tile([128, NH * HD1], bf16, name=f"vext{i}")
                    for i in range(8)]
            attn_b = [pp.tile([128, S], bf16, name=f"attnb{i}") for i in range(8)]

            roped = {}        # (pss, ch) -> rope'd tile (rolling window)

            # ------------------------- emitters ----------------------------
            def emit_qk_mms(ch, ps):
                if ch in (0, 8):
                    wt, col = wqkA_sb, (0 if ch == 0 else 128)
                else:
                    wt = wqkB_sb
                    col = (2 * ((ch % 8) - 1) + (0 if ch < 8 else 1)) * 128
                for dc in range(8):
                    for n in range(2):
                        nc.tensor.matmul(
                            ps[:, n * 512:(n + 1) * 512],
                            wt[dc][:, col:col + 128],
                            xT_sb[dc][:, n * 512:(n + 1) * 512],
                            start=(dc == 0), stop=(dc == 7))

            def emit_qk_swap(ch, ps):
                qk = wk.tile([128, S], bf16, tag="qk", bufs=1, name=f"qk{ch}")
                nc.vector.tensor_copy(qk[:], ps[:])
                sw = wk.tile([128, S], bf16, tag="sw", bufs=1, name=f"sw{ch}")
                for hh in range(2):
                    for f in range(2):
                        o0 = hh * 64 + f * 32
                        i0 = hh * 64 + (1 - f) * 32
                        nc.vector.tensor_copy(sw[o0:o0 + 32, :],
                                              qk[i0:i0 + 32, :])
                return qk, sw

            def emit_rope(ch, qk, sw, pss):
                rp = wk.tile([128, S], bf16, tag="roped", bufs=8,
                             name=f"rp{ch}_{pss}")
                bb = wk.tile([128, S], bf16, tag="rb", bufs=1,
                             name=f"rb{ch}_{pss}")
                nc.vector.tensor_mul(rp[:], qk[:], trigC_t[pss][:])
                nc.vector.tensor_mul(bb[:], sw[:], trigS_t[pss][:])
                nc.vector.tensor_add(rp[:], rp[:], bb[:])
                roped[(pss, ch)] = rp

            def emit_qk_chunk(ch):
                """Projection burst (16 matmuls) + evac + rotate-half + RoPE
                for one 128-row chunk of Q or K."""
                ps = pg.tile([128, S], f32, tag="big", bufs=2, name=f"qp{ch}")
                emit_qk_mms(ch, ps)
                qk, sw = emit_qk_swap(ch, ps)
                for pss in range(2):
                    emit_rope(ch, qk, sw, pss)

            def emit_v_block(sc):
                """V projection burst (16 matmuls) + vext evac for one
                128-row s-block."""
                ps = pg.tile([128, S], f32, tag="big", bufs=2, name=f"vp{sc}")
                for dc in range(8):
                    for n in range(2):
                        nc.tensor.matmul(
                            ps[:, n * 512:(n + 1) * 512],
                            xT_sb[dc][:, sc * 128:(sc + 1) * 128],
                            wvT_sb[dc][:, n * 512:(n + 1) * 512],
                            start=(dc == 0), stop=(dc == 7))
                vv = vext[sc][:].rearrange("p (h e) -> p h e", e=HD1)
                nc.vector.tensor_copy(
                    vv[:, :, 1:HD1],
                    ps[:].rearrange("p (h e) -> p h e", e=HD))
                nc.vector.memset(vv[:, :, 0:1], 2.0)

            es_tiles = {}     # (step, kc, n) -> es tile
            pv_tiles = {}     # step -> [pvE, pvO]

            def emit_scores(s, kc):
                cc, pss = s // 2, s % 2
                q1 = roped[(pss, cc)]
                k1 = roped[(pss, 8 + cc)]
                for n in range(2):
                    scp = pg.tile([128, S], f32, tag="big", bufs=2,
                                  name=f"scp{s}_{kc}_{n}")
                    for g, hh in ((0, 0), (1, 64)):
                        nc.tensor.matmul(
                            scp[:, g * 512:(g + 1) * 512],
                            k1[hh:hh + 64, kc * 128:(kc + 1) * 128],
                            q1[hh:hh + 64, n * 512:(n + 1) * 512],
                            start=True, stop=True)
                    es = wk.tile([128, S], bf16, tag="es", bufs=14,
                                 name=f"es{s}_{kc}_{n}")
                    nc.scalar.activation(es[:], scp[:], EXP, scale=0.125)
                    es_tiles[(s, kc, n)] = es

            def emit_pv(s, kc):
                if kc == 0:
                    pv_tiles[s] = [pg.tile([HD1, S], f32, tag="pv", bufs=2,
                                           name=f"pv{s}_{g}")
                                   for g in range(2)]
                cc = s // 2
                pvps = pv_tiles[s]
                for n in range(2):
                    es = es_tiles.pop((s, kc, n))
                    for g in range(2):
                        h = 2 * cc + g
                        nc.tensor.matmul(
                            pvps[g][:, n * 512:(n + 1) * 512],
                            vext[kc][:, h * HD1:(h + 1) * HD1],
                            es[:, g * 512:(g + 1) * 512],
                            start=(kc == 0), stop=(kc == 7))

            ct_store = {}     # g -> ct tile of current cc's pass 0

            def emit_evac(s):
                """Free step s's PV PSUM fast (one head via gpsimd copy, one
                via DVE copy, in parallel), then normalize on SBUF; on
                pss==1 combine passes and DMA into attn_b."""
                cc, pss = s // 2, s % 2
                pvps = pv_tiles.pop(s)
                pv_sbs = []
                for g in range(2):
                    pv_sb = wk.tile([HD1, S], f32, tag="pvsb", bufs=3,
                                    name=f"pvsb{s}_{g}")
                    nc.vector.tensor_copy(pv_sb[:], pvps[g][:])
                    pv_sbs.append(pv_sb)
                for g in range(2):
                    pv_sb = pv_sbs[g]
                    nc.vector.reciprocal_approx_fast(pv_sb[0:1, :],
                                                     pv_sb[0:1, :])
                    bc = wk.tile([HD1, S], f32, tag="bc", bufs=2,
                                 name=f"bc{s}_{g}")
                    nc.gpsimd.partition_broadcast(bc[:, :], pv_sb[0:1, :],
                                                  channels=HD1)
                    ct = wk.tile([HD1, S], bf16, tag=f"ct{g}", bufs=2,
                                 name=f"ct{s}_{g}")
                    nc.vector.tensor_mul(ct[:], pv_sb[:], bc[:])
                    if pss == 0:
                        ct_store[g] = ct
                    else:
                        ah = wk.tile([HD1, S], bf16, tag="ah", bufs=2,
                                     name=f"ah{s}_{g}")
                        nc.vector.tensor_add(ah[:], ct_store[g][:], ct[:])
                        nc.gpsimd.dma_start(attn_b[cc][g * 64:g * 64 + 64, :],
                                            ah[1:HD1, :])

            # ------------------------- pre-loop ----------------------------
            emit_qk_chunk(0)
            emit_qk_chunk(8)

            # projection stream: (clock, action); V blocks early (PV(0)
            # needs vext[kc] from clock 7+kc), qk chunks for cc at steps
            # 2cc-2 (clocks 16cc-15 / 16cc-11).
            proj_sched = {}
            vclocks = [4, 5, 6, 7, 8, 9, 10, 11]
            for sc in range(8):
                proj_sched.setdefault(vclocks[sc], []).append(("v", sc))
            for cc in range(1, 8):
                proj_sched.setdefault(16 * (cc - 1) + 1, []).append(("qk", cc))
                proj_sched.setdefault(16 * (cc - 1) + 9, []).append(("qk", 8 + cc))

            # ------------------------- main loop ---------------------------
            PV_OFF = 7        # pv for clock t runs at clock t + PV_OFF
            NSTEP = 16
            for c in range(NSTEP * 8 + PV_OFF):
                s, kc = c // 8, c % 8
                if s < NSTEP:
                    emit_scores(s, kc)
                t = c - PV_OFF
                if t >= 0:
                    emit_pv(t // 8, t % 8)
                    if t % 8 == 7:
                        emit_evac(t // 8)
                for act, arg in proj_sched.get(c, []):
                    if act == "v":
                        emit_v_block(arg)
                    else:
                        emit_qk_chunk(arg)

            # ---------------------- output projection ----------------------
            # cc 0-6 contributions can run while the last head-pair is still
            # draining; only the cc==7 matmul waits on the final attn_b strip.
            def oproj_partial(sc, op):
                for cc in range(7):
                    for n in range(2):
                        nc.tensor.matmul(
                            op[:, n * 512:(n + 1) * 512],
                            attn_b[cc][:, sc * 128:(sc + 1) * 128],
                            woT_sb[cc][:, n * 512:(n + 1) * 512],
                            start=(cc == 0), stop=False)

            def oproj_finish(sc, op):
                for n in range(2):
                    nc.tensor.matmul(
                        op[:, n * 512:(n + 1) * 512],
                        attn_b[7][:, sc * 128:(sc + 1) * 128],
                        woT_sb[7][:, n * 512:(n + 1) * 512],
                        start=False, stop=True)
                for q in range(2):
                    ob = wk.tile([128, 512], f32, tag="es", bufs=14,
                                 name=f"ob{sc}_{q}")
                    nc.vector.tensor_copy(ob[:], op[:, q * 512:(q + 1) * 512])
                    for r in range(2):
                        nc.sync.dma_start(
                            out_d[sc * 128:(sc + 1) * 128,
                                  q * 512 + r * 256:q * 512 + (r + 1) * 256],
                            ob[:, r * 256:(r + 1) * 256])

            op_t = {}
            for pair in range(4):
                for sc in (2 * pair, 2 * pair + 1):
                    op_t[sc] = pg.tile([128, DM], f32, tag="big", bufs=2,
                                       name=f"op{sc}")
                    oproj_partial(sc, op_t[sc])
                if pair == 0:
                    # keep the PE warm while the final head-pair's evac
                    # chain produces attn_b[7] (garbage matmuls, never read)
                    dmy = pg.tile([HD1, S], f32, tag="pv", bufs=2,
                                  name="dummy_ps")
                    for _ in range(12):
                        nc.tensor.matmul(dmy[0:HD1, 0:512],
                                         trigC_t[0][:, 0:HD1],
                                         trigC_t[0][:, 0:512],
                                         start=True, stop=True)
                for sc in (2 * pair, 2 * pair + 1):
                    oproj_finish(sc, op_t[sc])

    nc.compile()
    _CACHE[key] = nc
    return nc


def _prep_inputs(hidden_states, cos, sin, w_qkv, w_o):
    bf = ml_dtypes.bfloat16
    xT = np.ascontiguousarray(
        hidden_states.transpose(0, 2, 1)).astype(bf)          # [B, DM, S]
    wqkT = np.ascontiguousarray(w_qkv[:2 * DM].T).astype(bf)  # [DM, 2DM]
    wqkA = np.ascontiguousarray(
        np.concatenate([wqkT[:, 0:128], wqkT[:, 1024:1152]], axis=1))
    bcols = []
    for c in range(1, 8):
        bcols.append(wqkT[:, c * 128:(c + 1) * 128])
        bcols.append(wqkT[:, (8 + c) * 128:(9 + c) * 128])
    wqkB = np.ascontiguousarray(np.concatenate(bcols, axis=1))
    wvT = np.ascontiguousarray(w_qkv[2 * DM:].T).astype(bf)   # [DM, DM]
    woT = np.ascontiguousarray(w_o.T).astype(bf)              # [DM, DM]

    idx = np.arange(S).reshape(32, 32).T.reshape(-1)
    d = np.arange(128) % HD
    sign = np.where(d < 32, -1.0, 1.0).astype(np.float32)
    trigC = np.stack([
        np.ascontiguousarray(cos[:, d].T),
        np.ascontiguousarray(cos[idx][:, d].T),
    ]).astype(bf)                                             # [2, 128, S]
    trigS = np.stack([
        np.ascontiguousarray(sin[:, d].T) * sign[:, None],
        np.ascontiguousarray(sin[idx][:, d].T) * sign[:, None],
    ]).astype(bf)
    shared = {"wqkA": wqkA, "wqkB": wqkB, "wvT": wvT, "woT": woT,
              "trigC": trigC, "trigS": trigS}
    return [{"xT": np.ascontiguousarray(xT[b]), **shared} for b in range(B)]


def _install_ntff_hook():
    import sys, types
    if "antenv.axon_hooks" in sys.modules:
        return
    try:
        from trn_agent_boot.trn_boot import _ntff_profile_via_ctypes
        hook = _ntff_profile_via_ctypes('/opt/axon/libaxon_pjrt.so')
    except Exception:
        hook = None
    mod = types.ModuleType("antenv.axon_hooks")
    mod.get_axon_ntff_profile_hook = lambda: hook
    mod.set_axon_ntff_profile_hook = lambda h: None
    sys.modules["antenv.axon_hooks"] = mod


def kernel(hidden_states, cos, sin, w_qkv, w_o, _trace=False, _tmpdir=None):
    from concourse import bass_utils
    if _trace:
        _install_ntff_hook()
    nc = _build()
    in_maps = _prep_inputs(np.asarray(hidden_states, np.float32),
                           np.asarray(cos, np.float32),
                           np.asarray(sin, np.float32),
                           np.asarray(w_qkv, np.float32),
                           np.asarray(w_o, np.float32))
    res = bass_utils.run_bass_kernel_spmd(
        nc, in_maps, core_ids=list(range(NC)),
        trace=_trace, tmpdir=_tmpdir)
    out = np.stack([np.asarray(res.results[b]["out"], np.float32)
                    for b in range(B)])
    kernel.last_exec_time_ns = res.exec_time_ns
    return out


# revision 22
# speedup vs baseline: 1.0010x; 1.0010x over previous
"""Dual-RoPE attention block (B=8, S=1024, 16 heads x 64) on 8 NeuronCores.

Sharding: data-parallel over batch, one batch element per core.

v2: software-pipelined emission so the Scalar engine (exp softmax, the
~324us/core floor) runs continuously from ~15us, and the PE never idles
long enough to drop out of the HAM 8/8 clock state.

Per-core dataflow (all matmuls bf16 inputs, fp32 PSUM accumulation):
  - Q,K projected in head-transposed layout [c, s]; rotate_half via
    partition-swapped SBUF strip DMAs; RoPE = 3 vector ops per tile with
    presigned sin rows.
  - V projected in row layout [s, d]; each head's 65-column block in vext
    is [2.0-const | v], so the PV matmul's row 0 yields 2*sum_k(exp) and
    pass-averaging folds into normalization.
  - scores matmul pairs use 64-row PE tiles (auto tile_position) so both
    heads' scores stream concurrently through the array.
  - PV PSUM evacuated by strip DMAs (not DVE) so the PSUM slots recycle
    off the vector engine's critical path; normalization (reciprocal,
    gpsimd broadcast, multiply, pass-combine) runs downstream on SBUF.
  - emission interleaves per kc-clock: scores pair, 2 exps, previous
    step's PV, and occasional projection bursts for the next head-pair.
"""

import numpy as np
import ml_dtypes

B, S, DM = 8, 1024, 1024
NH, HD = 16, 64
HD1 = HD + 1
NC = 8                # cores

_CACHE = {}


def _build():
    key = ("v2",)
    if key in _CACHE:
        return _CACHE[key]
    from concourse import bacc, mybir
    import concourse.tile as tile

    f32 = mybir.dt.float32
    bf16 = mybir.dt.bfloat16
    EXP = mybir.ActivationFunctionType.Exp

    nc = bacc.Bacc("TRN2", target_bir_lowering=False, debug=False,
                   enable_asserts=False, num_devices=NC)

    xT_d = nc.dram_tensor("xT", [DM, S], bf16, kind="ExternalInput").ap()
    wqkA_d = nc.dram_tensor("wqkA", [DM, 256], bf16, kind="ExternalInput").ap()
    wqkB_d = nc.dram_tensor("wqkB", [DM, 2 * DM - 256], bf16,
                            kind="ExternalInput").ap()
    wvT_d = nc.dram_tensor("wvT", [DM, DM], bf16, kind="ExternalInput").ap()
    woT_d = nc.dram_tensor("woT", [DM, DM], bf16, kind="ExternalInput").ap()
    trigC_d = nc.dram_tensor("trigC", [2, 128, S], bf16, kind="ExternalInput").ap()
    trigS_d = nc.dram_tensor("trigS", [2, 128, S], bf16, kind="ExternalInput").ap()
    out_d = nc.dram_tensor("out", [S, DM], f32, kind="ExternalOutput").ap()

    with tile.TileContext(nc) as tc:
        with (
            tc.tile_pool(name="persist", bufs=1) as pp,
            tc.tile_pool(name="work", bufs=1) as wk,
            tc.tile_pool(name="psg", bufs=1, space="PSUM") as pg,
        ):
            # ---------------- persistent tiles + input DMAs ----------------
            trigC_t = [pp.tile([128, S], bf16, name=f"trigC{p}") for p in range(2)]
            trigS_t = [pp.tile([128, S], bf16, name=f"trigS{p}") for p in range(2)]
            for p in range(2):
                nc.sync.dma_start(trigC_t[p][:], trigC_d[p])
                nc.sync.dma_start(trigS_t[p][:], trigS_d[p])

            xT_sb = [pp.tile([128, S], bf16, name=f"xT{i}") for i in range(8)]
            wqkA_sb = [pp.tile([128, 256], bf16, name=f"wqkA{i}")
                       for i in range(8)]
            wqkB_sb = [pp.tile([128, 2 * DM - 256], bf16, name=f"wqkB{i}")
                       for i in range(8)]
            wvT_sb = [pp.tile([128, DM], bf16, name=f"wv{i}") for i in range(8)]
            woT_sb = [pp.tile([128, DM], bf16, name=f"woT{i}") for i in range(8)]
            # DMA priority: xT + first-chunk weights, then V weights, then
            # the remaining qk weights, output weights last.
            for i in range(8):
                nc.sync.dma_start(xT_sb[i][:], xT_d[i * 128:(i + 1) * 128, :])
                nc.sync.dma_start(wqkA_sb[i][:], wqkA_d[i * 128:(i + 1) * 128, :])
            for i in range(8):
                nc.sync.dma_start(wvT_sb[i][:], wvT_d[i * 128:(i + 1) * 128, :])
            for i in range(8):
                nc.sync.dma_start(wqkB_sb[i][:], wqkB_d[i * 128:(i + 1) * 128, :])
            for i in range(8):
                nc.sync.dma_start(woT_sb[i][:], woT_d[i * 128:(i + 1) * 128, :])

            vext = [pp.tile([128, NH * HD1], bf16, name=f"vext{i}")
                    for i in range(8)]
            attn_b = [pp.tile([128, S], bf16, name=f"attnb{i}") for i in range(8)]

            roped = {}        # (pss, ch) -> rope'd tile (rolling window)

            # ------------------------- emitters ----------------------------
            def emit_qk_mms(ch, ps):
                if ch in (0, 8):
                    wt, col = wqkA_sb, (0 if ch == 0 else 128)
                else:
                    wt = wqkB_sb
                    col = (2 * ((ch % 8) - 1) + (0 if ch < 8 else 1)) * 128
                for dc in range(8):
                    for n in range(2):
                        nc.tensor.matmul(
                            ps[:, n * 512:(n + 1) * 512],
                            wt[dc][:, col:col + 128],
                            xT_sb[dc][:, n * 512:(n + 1) * 512],
                            start=(dc == 0), stop=(dc == 7))

            def emit_qk_swap(ch, ps):
                qk = wk.tile([128, S], bf16, tag="qk", bufs=1, name=f"qk{ch}")
                nc.vector.tensor_copy(qk[:], ps[:])
                sw = wk.tile([128, S], bf16, tag="sw", bufs=1, name=f"sw{ch}")
                for hh in range(2):
                    for f in range(2):
                        o0 = hh * 64 + f * 32
                        i0 = hh * 64 + (1 - f) * 32
                        nc.vector.tensor_copy(sw[o0:o0 + 32, :],
                                              qk[i0:i0 + 32, :])
                return qk, sw

            def emit_rope(ch, qk, sw, pss):
                rp = wk.tile([128, S], bf16, tag="roped", bufs=8,
                             name=f"rp{ch}_{pss}")
                bb = wk.tile([128, S], bf16, tag="rb", bufs=1,
                             name=f"rb{ch}_{pss}")
                nc.vector.tensor_mul(rp[:], qk[:], trigC_t[pss][:])
                nc.vector.tensor_mul(bb[:], sw[:], trigS_t[pss][:])
                nc.vector.tensor_add(rp[:], rp[:], bb[:])
                roped[(pss, ch)] = rp

            def emit_qk_chunk(ch):
                """Projection burst (16 matmuls) + evac + rotate-half + RoPE
                for one 128-row chunk of Q or K."""
                ps = pg.tile([128, S], f32, tag="big", bufs=2, name=f"qp{ch}")
                emit_qk_mms(ch, ps)
                qk, sw = emit_qk_swap(ch, ps)
                for pss in range(2):
                    emit_rope(ch, qk, sw, pss)

            def emit_v_block(sc):
                """V projection burst (16 matmuls) + vext evac for one
                128-row s-block."""
                ps = pg.tile([128, S], f32, tag="big", bufs=2, name=f"vp{sc}")
                for dc in range(8):
                    for n in range(2):
                        nc.tensor.matmul(
                            ps[:, n * 512:(n + 1) * 512],
                            xT_sb[dc][:, sc * 128:(sc + 1) * 128],
                            wvT_sb[dc][:, n * 512:(n + 1) * 512],
                            start=(dc == 0), stop=(dc == 7))
                vv = vext[sc][:].rearrange("p (h e) -> p h e", e=HD1)
                nc.vector.tensor_copy(
                    vv[:, :, 1:HD1],
                    ps[:].rearrange("p (h e) -> p h e", e=HD))
                nc.vector.memset(vv[:, :, 0:1], 2.0)

            es_tiles = {}     # (step, kc, n) -> es tile
            pv_tiles = {}     # step -> [pvE, pvO]

            def emit_scores(s, kc):
                cc, pss = s // 2, s % 2
                q1 = roped[(pss, cc)]
                k1 = roped[(pss, 8 + cc)]
                for n in range(2):
                    scp = pg.tile([128, S], f32, tag="big", bufs=2,
                                  name=f"scp{s}_{kc}_{n}")
                    for g, hh in ((0, 0), (1, 64)):
                        nc.tensor.matmul(
                            scp[:, g * 512:(g + 1) * 512],
                            k1[hh:hh + 64, kc * 128:(kc + 1) * 128],
                            q1[hh:hh + 64, n * 512:(n + 1) * 512],
                            start=True, stop=True)
                    es = wk.tile([128, S], bf16, tag="es", bufs=14,
                                 name=f"es{s}_{kc}_{n}")
                    nc.scalar.activation(es[:], scp[:], EXP, scale=0.125)
                    es_tiles[(s, kc, n)] = es

            def emit_pv(s, kc):
                if kc == 0:
                    pv_tiles[s] = [pg.tile([HD1, S], f32, tag="pv", bufs=2,
                                           name=f"pv{s}_{g}")
                                   for g in range(2)]
                cc = s // 2
                pvps = pv_tiles[s]
                for n in range(2):
                    es = es_tiles.pop((s, kc, n))
                    for g in range(2):
                        h = 2 * cc + g
                        nc.tensor.matmul(
                            pvps[g][:, n * 512:(n + 1) * 512],
                            vext[kc][:, h * HD1:(h + 1) * HD1],
                            es[:, g * 512:(g + 1) * 512],
                            start=(kc == 0), stop=(kc == 7))

            ct_store = {}     # g -> ct tile of current cc's pass 0

            def emit_evac(s):
                """Free step s's PV PSUM fast (one head via gpsimd copy, one
                via DVE copy, in parallel), then normalize on SBUF; on
                pss==1 combine passes and DMA into attn_b."""
                cc, pss = s // 2, s % 2
                pvps = pv_tiles.pop(s)
                pv_sbs = []
                for g in range(2):
                    pv_sb = wk.tile([HD1, S], f32, tag="pvsb", bufs=3,
                                    name=f"pvsb{s}_{g}")
                    nc.vector.tensor_copy(pv_sb[:], pvps[g][:])
                    pv_sbs.append(pv_sb)
                for g in range(2):
                    pv_sb = pv_sbs[g]
                    nc.vector.reciprocal_approx_fast(pv_sb[0:1, :],
                                                     pv_sb[0:1, :])
                    rec = wk.tile([1, S], bf16, tag="rec", bufs=2,
                                  name=f"rc{s}_{g}")
                    with nc.allow_low_precision(reason="bf16 recip of sums"):
                        nc.vector.tensor_copy(rec[0:1, :], pv_sb[0:1, :])
                    bc = wk.tile([HD1, S], bf16, tag="bc", bufs=2,
                                 name=f"bc{s}_{g}")
                    nc.gpsimd.partition_broadcast(bc[:, :], rec[0:1, :],
                                                  channels=HD1)
                    ct = wk.tile([HD1, S], bf16, tag=f"ct{g}", bufs=2,
                                 name=f"ct{s}_{g}")
                    nc.vector.tensor_mul(ct[:], pv_sb[:], bc[:])
                    if pss == 0:
                        ct_store[g] = ct
                    else:
                        ah = wk.tile([HD1, S], bf16, tag="ah", bufs=2,
                                     name=f"ah{s}_{g}")
                        nc.vector.tensor_add(ah[:], ct_store[g][:], ct[:])
                        nc.gpsimd.dma_start(attn_b[cc][g * 64:g * 64 + 64, :],
                                            ah[1:HD1, :])

            # ------------------------- pre-loop ----------------------------
            emit_qk_chunk(0)
            emit_qk_chunk(8)

            # projection stream: (clock, action); V blocks early (PV(0)
            # needs vext[kc] from clock 7+kc), qk chunks for cc at steps
            # 2cc-2 (clocks 16cc-15 / 16cc-11).
            proj_sched = {}
            vclocks = [4, 5, 6, 7, 8, 9, 10, 11]
            for sc in range(8):
                proj_sched.setdefault(vclocks[sc], []).append(("v", sc))
            for cc in range(1, 8):
                proj_sched.setdefault(16 * (cc - 1) + 1, []).append(("qk", cc))
                proj_sched.setdefault(16 * (cc - 1) + 9, []).append(("qk", 8 + cc))

            # ------------------------- main loop ---------------------------
            PV_OFF = 7        # pv for clock t runs at clock t + PV_OFF
            NSTEP = 16
            for c in range(NSTEP * 8 + PV_OFF):
                s, kc = c // 8, c % 8
                if s < NSTEP:
                    emit_scores(s, kc)
                t = c - PV_OFF
                if t >= 0:
                    emit_pv(t // 8, t % 8)
                    if t % 8 == 7:
                        emit_evac(t // 8)
                for act, arg in proj_sched.get(c, []):
                    if act == "v":
                        emit_v_block(arg)
                    else:
                        emit_qk_chunk(arg)

            # ---------------------- output projection ----------------------
            # cc 0-6 contributions can run while the last head-pair is still
            # draining; only the cc==7 matmul waits on the final attn_b strip.
            def oproj_partial(sc, op):
                for cc in range(7):
                    for n in range(2):
                        nc.tensor.matmul(
                            op[:, n * 512:(n + 1) * 512],
                            attn_b[cc][:, sc * 128:(sc + 1) * 128],
                            woT_sb[cc][:, n * 512:(n + 1) * 512],
                            start=(cc == 0), stop=False)

            def oproj_finish(sc, op):
                for n in range(2):
                    nc.tensor.matmul(
                        op[:, n * 512:(n + 1) * 512],
                        attn_b[7][:, sc * 128:(sc + 1) * 128],
                        woT_sb[7][:, n * 512:(n + 1) * 512],
                        start=False, stop=True)
                for q in range(2):
                    ob = wk.tile([128, 512], f32, tag="es", bufs=14,
                                 name=f"ob{sc}_{q}")
                    nc.vector.tensor_copy(ob[:], op[:, q * 512:(q + 1) * 512])
                    for r in range(2):
                        nc.sync.dma_start(
                            out_d[sc * 128:(sc + 1) * 128,
                                  q * 512 + r * 256:q * 512 + (r + 1) * 256],
                            ob[:, r * 256:(r + 1) * 256])

            op_t = {}
            for pair in range(4):
                for sc in (2 * pair, 2 * pair + 1):
                    op_t[sc] = pg.tile([128, DM], f32, tag="big", bufs=2,
                                       name=f"op{sc}")
                    oproj_partial(sc, op_t[sc])
                if pair == 0:
                    # keep the PE warm while the final head-pair's evac
                    # chain produces attn_b[7] (garbage matmuls, never read)
                    dmy = pg.tile([HD1, S], f32, tag="pv", bufs=2,
                                  name="dummy_ps")
                    for _ in range(12):
                        nc.tensor.matmul(dmy[0:HD1, 0:512],
                                         trigC_t[0][:, 0:HD1],
                                         trigC_t[0][:, 0:512],
                                         start=True, stop=True)
                for sc in (2 * pair, 2 * pair + 1):
                    oproj_finish(sc, op_t[sc])

    nc.compile()
    _CACHE[key] = nc
    return nc


def _prep_inputs(hidden_states, cos, sin, w_qkv, w_o):
    bf = ml_dtypes.bfloat16
    xT = np.ascontiguousarray(
        hidden_states.transpose(0, 2, 1)).astype(bf)          # [B, DM, S]
    wqkT = np.ascontiguousarray(w_qkv[:2 * DM].T).astype(bf)  # [DM, 2DM]
    wqkA = np.ascontiguousarray(
        np.concatenate([wqkT[:, 0:128], wqkT[:, 1024:1152]], axis=1))
    bcols = []
    for c in range(1, 8):
        bcols.append(wqkT[:, c * 128:(c + 1) * 128])
        bcols.append(wqkT[:, (8 + c) * 128:(9 + c) * 128])
    wqkB = np.ascontiguousarray(np.concatenate(bcols, axis=1))
    wvT = np.ascontiguousarray(w_qkv[2 * DM:].T).astype(bf)   # [DM, DM]
    woT = np.ascontiguousarray(w_o.T).astype(bf)              # [DM, DM]

    idx = np.arange(S).reshape(32, 32).T.reshape(-1)
    d = np.arange(128) % HD
    sign = np.where(d < 32, -1.0, 1.0).astype(np.float32)
    trigC = np.stack([
        np.ascontiguousarray(cos[:, d].T),
        np.ascontiguousarray(cos[idx][:, d].T),
    ]).astype(bf)                                             # [2, 128, S]
    trigS = np.stack([
        np.ascontiguousarray(sin[:, d].T) * sign[:, None],
        np.ascontiguousarray(sin[idx][:, d].T) * sign[:, None],
    ]).astype(bf)
    shared = {"wqkA": wqkA, "wqkB": wqkB, "wvT": wvT, "woT": woT,
              "trigC": trigC, "trigS": trigS}
    return [{"xT": np.ascontiguousarray(xT[b]), **shared} for b in range(B)]


def _install_ntff_hook():
    import sys, types
    if "antenv.axon_hooks" in sys.modules:
        return
    try:
        from trn_agent_boot.trn_boot import _ntff_profile_via_ctypes
        hook = _ntff_profile_via_ctypes('/opt/axon/libaxon_pjrt.so')
    except Exception:
        hook = None
    mod = types.ModuleType("antenv.axon_hooks")
    mod.get_axon_ntff_profile_hook = lambda: hook
    mod.set_axon_ntff_profile_hook = lambda h: None
    sys.modules["antenv.axon_hooks"] = mod


def kernel(hidden_states, cos, sin, w_qkv, w_o, _trace=False, _tmpdir=None):
    from concourse import bass_utils
    if _trace:
        _install_ntff_hook()
    nc = _build()
    in_maps = _prep_inputs(np.asarray(hidden_states, np.float32),
                           np.asarray(cos, np.float32),
                           np.asarray(sin, np.float32),
                           np.asarray(w_qkv, np.float32),
                           np.asarray(w_o, np.float32))
    res = bass_utils.run_bass_kernel_spmd(
        nc, in_maps, core_ids=list(range(NC)),
        trace=_trace, tmpdir=_tmpdir)
    out = np.stack([np.asarray(res.results[b]["out"], np.float32)
                    for b in range(B)])
    kernel.last_exec_time_ns = res.exec_time_ns
    return out


# revision 23
# speedup vs baseline: 1.0205x; 1.0194x over previous
"""Dual-RoPE attention block (B=8, S=1024, 16 heads x 64) on 8 NeuronCores.

Sharding: data-parallel over batch, one batch element per core.

v2: software-pipelined emission so the Scalar engine (exp softmax, the
~324us/core floor) runs continuously from ~15us, and the PE never idles
long enough to drop out of the HAM 8/8 clock state.

Per-core dataflow (all matmuls bf16 inputs, fp32 PSUM accumulation):
  - Q,K projected in head-transposed layout [c, s]; rotate_half via
    partition-swapped SBUF strip DMAs; RoPE = 3 vector ops per tile with
    presigned sin rows.
  - V projected in row layout [s, d]; each head's 65-column block in vext
    is [2.0-const | v], so the PV matmul's row 0 yields 2*sum_k(exp) and
    pass-averaging folds into normalization.
  - scores matmul pairs use 64-row PE tiles (auto tile_position) so both
    heads' scores stream concurrently through the array.
  - PV PSUM evacuated by strip DMAs (not DVE) so the PSUM slots recycle
    off the vector engine's critical path; normalization (reciprocal,
    gpsimd broadcast, multiply, pass-combine) runs downstream on SBUF.
  - emission interleaves per kc-clock: scores pair, 2 exps, previous
    step's PV, and occasional projection bursts for the next head-pair.
"""

import numpy as np
import ml_dtypes

B, S, DM = 8, 1024, 1024
NH, HD = 16, 64
HD1 = HD + 1
NC = 8                # cores

_CACHE = {}


def _build():
    key = ("v2",)
    if key in _CACHE:
        return _CACHE[key]
    from concourse import bacc, mybir
    import concourse.tile as tile

    f32 = mybir.dt.float32
    bf16 = mybir.dt.bfloat16
    EXP = mybir.ActivationFunctionType.Exp

    nc = bacc.Bacc("TRN2", target_bir_lowering=False, debug=False,
                   enable_asserts=False, num_devices=NC)

    xT_d = nc.dram_tensor("xT", [DM, S], bf16, kind="ExternalInput").ap()
    wqkA_d = nc.dram_tensor("wqkA", [DM, 256], bf16, kind="ExternalInput").ap()
    wqkB_d = nc.dram_tensor("wqkB", [DM, 2 * DM - 256], bf16,
                            kind="ExternalInput").ap()
    wvT_d = nc.dram_tensor("wvT", [DM, DM], bf16, kind="ExternalInput").ap()
    woT_d = nc.dram_tensor("woT", [DM, DM], bf16, kind="ExternalInput").ap()
    trigC_d = nc.dram_tensor("trigC", [2, 128, S], bf16, kind="ExternalInput").ap()
    trigS_d = nc.dram_tensor("trigS", [2, 128, S], bf16, kind="ExternalInput").ap()
    out_d = nc.dram_tensor("out", [S, DM], f32, kind="ExternalOutput").ap()

    with tile.TileContext(nc) as tc:
        with (
            tc.tile_pool(name="persist", bufs=1) as pp,
            tc.tile_pool(name="work", bufs=1) as wk,
            tc.tile_pool(name="psg", bufs=1, space="PSUM") as pg,
        ):
            # ---------------- persistent tiles + input DMAs ----------------
            trigC_t = [pp.tile([128, S], bf16, name=f"trigC{p}") for p in range(2)]
            trigS_t = [pp.tile([128, S], bf16, name=f"trigS{p}") for p in range(2)]
            xT_sb = [pp.tile([128, S], bf16, name=f"xT{i}") for i in range(8)]
            wqkA_sb = [pp.tile([128, 256], bf16, name=f"wqkA{i}")
                       for i in range(8)]
            wqkB_sb = [pp.tile([128, 2 * DM - 256], bf16, name=f"wqkB{i}")
                       for i in range(8)]
            wvT_sb = [pp.tile([128, DM], bf16, name=f"wv{i}") for i in range(8)]
            woT_sb = [pp.tile([128, DM], bf16, name=f"woT{i}") for i in range(8)]
            # DMA priority: xT + first-chunk weights, then V weights, then
            # the remaining qk weights, output weights last.
            for i in range(8):
                nc.sync.dma_start(xT_sb[i][:], xT_d[i * 128:(i + 1) * 128, :])
                nc.sync.dma_start(wqkA_sb[i][:], wqkA_d[i * 128:(i + 1) * 128, :])
            for p in range(2):
                nc.sync.dma_start(trigC_t[p][:], trigC_d[p])
                nc.sync.dma_start(trigS_t[p][:], trigS_d[p])
            for i in range(8):
                nc.sync.dma_start(wvT_sb[i][:], wvT_d[i * 128:(i + 1) * 128, :])
            for i in range(8):
                nc.sync.dma_start(wqkB_sb[i][:], wqkB_d[i * 128:(i + 1) * 128, :])
            for i in range(8):
                nc.sync.dma_start(woT_sb[i][:], woT_d[i * 128:(i + 1) * 128, :])

            vext = [pp.tile([128, NH * HD1], bf16, name=f"vext{i}")
                    for i in range(8)]
            attn_b = [pp.tile([128, S], bf16, name=f"attnb{i}") for i in range(8)]

            roped = {}        # (pss, ch) -> rope'd tile (rolling window)

            # ------------------------- emitters ----------------------------
            def emit_qk_mms(ch, ps):
                if ch in (0, 8):
                    wt, col = wqkA_sb, (0 if ch == 0 else 128)
                else:
                    wt = wqkB_sb
                    col = (2 * ((ch % 8) - 1) + (0 if ch < 8 else 1)) * 128
                for dc in range(8):
                    for n in range(2):
                        nc.tensor.matmul(
                            ps[:, n * 512:(n + 1) * 512],
                            wt[dc][:, col:col + 128],
                            xT_sb[dc][:, n * 512:(n + 1) * 512],
                            start=(dc == 0), stop=(dc == 7))

            def emit_qk_swap(ch, ps):
                qk = wk.tile([128, S], bf16, tag="qk", bufs=1, name=f"qk{ch}")
                nc.vector.tensor_copy(qk[:], ps[:])
                sw = wk.tile([128, S], bf16, tag="sw", bufs=1, name=f"sw{ch}")
                for hh in range(2):
                    for f in range(2):
                        o0 = hh * 64 + f * 32
                        i0 = hh * 64 + (1 - f) * 32
                        nc.vector.tensor_copy(sw[o0:o0 + 32, :],
                                              qk[i0:i0 + 32, :])
                return qk, sw

            def emit_rope(ch, qk, sw, pss):
                rp = wk.tile([128, S], bf16, tag="roped", bufs=8,
                             name=f"rp{ch}_{pss}")
                bb = wk.tile([128, S], bf16, tag="rb", bufs=1,
                             name=f"rb{ch}_{pss}")
                nc.vector.tensor_mul(rp[:], qk[:], trigC_t[pss][:])
                nc.vector.tensor_mul(bb[:], sw[:], trigS_t[pss][:])
                nc.vector.tensor_add(rp[:], rp[:], bb[:])
                roped[(pss, ch)] = rp

            def emit_qk_chunk(ch):
                """Projection burst (16 matmuls) + evac + rotate-half + RoPE
                for one 128-row chunk of Q or K."""
                ps = pg.tile([128, S], f32, tag="big", bufs=2, name=f"qp{ch}")
                emit_qk_mms(ch, ps)
                qk, sw = emit_qk_swap(ch, ps)
                for pss in range(2):
                    emit_rope(ch, qk, sw, pss)

            def emit_v_block(sc):
                """V projection burst (16 matmuls) + vext evac for one
                128-row s-block."""
                ps = pg.tile([128, S], f32, tag="big", bufs=2, name=f"vp{sc}")
                for dc in range(8):
                    for n in range(2):
                        nc.tensor.matmul(
                            ps[:, n * 512:(n + 1) * 512],
                            xT_sb[dc][:, sc * 128:(sc + 1) * 128],
                            wvT_sb[dc][:, n * 512:(n + 1) * 512],
                            start=(dc == 0), stop=(dc == 7))
                vv = vext[sc][:].rearrange("p (h e) -> p h e", e=HD1)
                nc.vector.tensor_copy(
                    vv[:, :, 1:HD1],
                    ps[:].rearrange("p (h e) -> p h e", e=HD))
                nc.vector.memset(vv[:, :, 0:1], 2.0)

            es_tiles = {}     # (step, kc, n) -> es tile
            pv_tiles = {}     # step -> [pvE, pvO]

            def emit_scores(s, kc):
                cc, pss = s // 2, s % 2
                q1 = roped[(pss, cc)]
                k1 = roped[(pss, 8 + cc)]
                for n in range(2):
                    scp = pg.tile([128, S], f32, tag="big", bufs=2,
                                  name=f"scp{s}_{kc}_{n}")
                    for g, hh in ((0, 0), (1, 64)):
                        nc.tensor.matmul(
                            scp[:, g * 512:(g + 1) * 512],
                            k1[hh:hh + 64, kc * 128:(kc + 1) * 128],
                            q1[hh:hh + 64, n * 512:(n + 1) * 512],
                            start=True, stop=True)
                    es = wk.tile([128, S], bf16, tag="es", bufs=14,
                                 name=f"es{s}_{kc}_{n}")
                    nc.scalar.activation(es[:], scp[:], EXP, scale=0.125)
                    es_tiles[(s, kc, n)] = es

            def emit_pv(s, kc):
                if kc == 0:
                    pv_tiles[s] = [pg.tile([HD1, S], f32, tag="pv", bufs=2,
                                           name=f"pv{s}_{g}")
                                   for g in range(2)]
                cc = s // 2
                pvps = pv_tiles[s]
                for n in range(2):
                    es = es_tiles.pop((s, kc, n))
                    for g in range(2):
                        h = 2 * cc + g
                        nc.tensor.matmul(
                            pvps[g][:, n * 512:(n + 1) * 512],
                            vext[kc][:, h * HD1:(h + 1) * HD1],
                            es[:, g * 512:(g + 1) * 512],
                            start=(kc == 0), stop=(kc == 7))

            ct_store = {}     # g -> ct tile of current cc's pass 0

            def emit_evac(s):
                """Free step s's PV PSUM fast (one head via gpsimd copy, one
                via DVE copy, in parallel), then normalize on SBUF; on
                pss==1 combine passes and DMA into attn_b."""
                cc, pss = s // 2, s % 2
                pvps = pv_tiles.pop(s)
                pv_sbs = []
                for g in range(2):
                    pv_sb = wk.tile([HD1, S], f32, tag="pvsb", bufs=3,
                                    name=f"pvsb{s}_{g}")
                    nc.vector.tensor_copy(pv_sb[:], pvps[g][:])
                    pv_sbs.append(pv_sb)
                for g in range(2):
                    pv_sb = pv_sbs[g]
                    nc.vector.reciprocal_approx_fast(pv_sb[0:1, :],
                                                     pv_sb[0:1, :])
                    rec = wk.tile([1, S], bf16, tag="rec", bufs=2,
                                  name=f"rc{s}_{g}")
                    with nc.allow_low_precision(reason="bf16 recip of sums"):
                        nc.vector.tensor_copy(rec[0:1, :], pv_sb[0:1, :])
                    bc = wk.tile([HD1, S], bf16, tag="bc", bufs=2,
                                 name=f"bc{s}_{g}")
                    nc.gpsimd.partition_broadcast(bc[:, :], rec[0:1, :],
                                                  channels=HD1)
                    ct = wk.tile([HD1, S], bf16, tag=f"ct{g}", bufs=2,
                                 name=f"ct{s}_{g}")
                    nc.vector.tensor_mul(ct[:], pv_sb[:], bc[:])
                    if pss == 0:
                        ct_store[g] = ct
                    else:
                        ah = wk.tile([HD1, S], bf16, tag="ah", bufs=2,
                                     name=f"ah{s}_{g}")
                        nc.vector.tensor_add(ah[:], ct_store[g][:], ct[:])
                        nc.gpsimd.dma_start(attn_b[cc][g * 64:g * 64 + 64, :],
                                            ah[1:HD1, :])

            # ------------------------- pre-loop ----------------------------
            emit_qk_chunk(0)
            emit_qk_chunk(8)

            # projection stream: (clock, action); V blocks early (PV(0)
            # needs vext[kc] from clock 7+kc), qk chunks for cc at steps
            # 2cc-2 (clocks 16cc-15 / 16cc-11).
            proj_sched = {}
            vclocks = [4, 5, 6, 7, 8, 9, 10, 11]
            for sc in range(8):
                proj_sched.setdefault(vclocks[sc], []).append(("v", sc))
            for cc in range(1, 8):
                proj_sched.setdefault(16 * (cc - 1) + 1, []).append(("qk", cc))
                proj_sched.setdefault(16 * (cc - 1) + 9, []).append(("qk", 8 + cc))

            # ------------------------- main loop ---------------------------
            PV_OFF = 7        # pv for clock t runs at clock t + PV_OFF
            NSTEP = 16
            for c in range(NSTEP * 8 + PV_OFF):
                s, kc = c // 8, c % 8
                if s < NSTEP:
                    emit_scores(s, kc)
                t = c - PV_OFF
                if t >= 0:
                    emit_pv(t // 8, t % 8)
                    if t % 8 == 7:
                        emit_evac(t // 8)
                for act, arg in proj_sched.get(c, []):
                    if act == "v":
                        emit_v_block(arg)
                    else:
                        emit_qk_chunk(arg)

            # ---------------------- output projection ----------------------
            # cc 0-6 contributions can run while the last head-pair is still
            # draining; only the cc==7 matmul waits on the final attn_b strip.
            def oproj_partial(sc, op):
                for cc in range(7):
                    for n in range(2):
                        nc.tensor.matmul(
                            op[:, n * 512:(n + 1) * 512],
                            attn_b[cc][:, sc * 128:(sc + 1) * 128],
                            woT_sb[cc][:, n * 512:(n + 1) * 512],
                            start=(cc == 0), stop=False)

            def oproj_finish(sc, op):
                for n in range(2):
                    nc.tensor.matmul(
                        op[:, n * 512:(n + 1) * 512],
                        attn_b[7][:, sc * 128:(sc + 1) * 128],
                        woT_sb[7][:, n * 512:(n + 1) * 512],
                        start=False, stop=True)
                for q in range(2):
                    ob = wk.tile([128, 512], f32, tag="es", bufs=14,
                                 name=f"ob{sc}_{q}")
                    nc.vector.tensor_copy(ob[:], op[:, q * 512:(q + 1) * 512])
                    for r in range(2):
                        nc.sync.dma_start(
                            out_d[sc * 128:(sc + 1) * 128,
                                  q * 512 + r * 256:q * 512 + (r + 1) * 256],
                            ob[:, r * 256:(r + 1) * 256])

            op_t = {}
            for wave in range(2):
                scs = range(4 * wave, 4 * wave + 4)
                for sc in scs:
                    tag = "big" if sc % 4 < 2 else "pv"
                    op_t[sc] = pg.tile([128, DM], f32, tag=tag, bufs=2,
                                       name=f"op{sc}")
                    oproj_partial(sc, op_t[sc])
                for sc in scs:
                    oproj_finish(sc, op_t[sc])

    nc.compile()
    _CACHE[key] = nc
    return nc


def _prep_inputs(hidden_states, cos, sin, w_qkv, w_o):
    bf = ml_dtypes.bfloat16
    xT = np.ascontiguousarray(
        hidden_states.transpose(0, 2, 1)).astype(bf)          # [B, DM, S]
    wqkT = np.ascontiguousarray(w_qkv[:2 * DM].T).astype(bf)  # [DM, 2DM]
    wqkA = np.ascontiguousarray(
        np.concatenate([wqkT[:, 0:128], wqkT[:, 1024:1152]], axis=1))
    bcols = []
    for c in range(1, 8):
        bcols.append(wqkT[:, c * 128:(c + 1) * 128])
        bcols.append(wqkT[:, (8 + c) * 128:(9 + c) * 128])
    wqkB = np.ascontiguousarray(np.concatenate(bcols, axis=1))
    wvT = np.ascontiguousarray(w_qkv[2 * DM:].T).astype(bf)   # [DM, DM]
    woT = np.ascontiguousarray(w_o.T).astype(bf)              # [DM, DM]

    idx = np.arange(S).reshape(32, 32).T.reshape(-1)
    d = np.arange(128) % HD
    sign = np.where(d < 32, -1.0, 1.0).astype(np.float32)
    trigC = np.stack([
        np.ascontiguousarray(cos[:, d].T),
        np.ascontiguousarray(cos[idx][:, d].T),
    ]).astype(bf)                                             # [2, 128, S]
    trigS = np.stack([
        np.ascontiguousarray(sin[:, d].T) * sign[:, None],
        np.ascontiguousarray(sin[idx][:, d].T) * sign[:, None],
    ]).astype(bf)
    shared = {"wqkA": wqkA, "wqkB": wqkB, "wvT": wvT, "woT": woT,
              "trigC": trigC, "trigS": trigS}
    return [{"xT": np.ascontiguousarray(xT[b]), **shared} for b in range(B)]


def _install_ntff_hook():
    import sys, types
    if "antenv.axon_hooks" in sys.modules:
        return
    try:
        from trn_agent_boot.trn_boot import _ntff_profile_via_ctypes
        hook = _ntff_profile_via_ctypes('/opt/axon/libaxon_pjrt.so')
    except Exception:
        hook = None
    mod = types.ModuleType("antenv.axon_hooks")
    mod.get_axon_ntff_profile_hook = lambda: hook
    mod.set_axon_ntff_profile_hook = lambda h: None
    sys.modules["antenv.axon_hooks"] = mod


def kernel(hidden_states, cos, sin, w_qkv, w_o, _trace=False, _tmpdir=None):
    from concourse import bass_utils
    if _trace:
        _install_ntff_hook()
    nc = _build()
    in_maps = _prep_inputs(np.asarray(hidden_states, np.float32),
                           np.asarray(cos, np.float32),
                           np.asarray(sin, np.float32),
                           np.asarray(w_qkv, np.float32),
                           np.asarray(w_o, np.float32))
    res = bass_utils.run_bass_kernel_spmd(
        nc, in_maps, core_ids=list(range(NC)),
        trace=_trace, tmpdir=_tmpdir)
    out = np.stack([np.asarray(res.results[b]["out"], np.float32)
                    for b in range(B)])
    kernel.last_exec_time_ns = res.exec_time_ns
    return out
